# revision 25
# baseline (speedup 1.0000x reference)
"""Trainium2 Bass kernel for nn_BboxRegressionLoss (topk_masking).

Math notes
----------
reference computes, with iou1ds = iou2ds reshaped [M, P] (mask2d all-ones):
    mask = scatter(top3_idx) | (iou1ds > 0.5)
    loss = |so + starts - tgt_s| + |eo + ends - tgt_e|     (per [M, P] element)
    out  = (loss * mask).sum() / mask.sum()

Key identity: if a row has >= TOPK elements with iou > 0.5, its top-TOPK
elements are all already inside the threshold mask, so mask == (iou > 0.5)
EXACTLY for that row. Per-row (per-target) counts are computed on device
(needed for mask.sum() anyway); host verifies the identity for every target
and falls back to an exact numpy replica otherwise.

VIDEO-MAJOR layout (v7) - eliminates the PE entirely
----------------------------------------------------
Previous structure replicated the K=32 so/eo source rows to 128 target
partitions with matmuls (PE measured ~47us busy - as hot as ACT/DVE).
Instead, partitions are now (video v, P-quarter j): 32 videos x 4 quarters.
Each partition holds so2[v, j*4096:(j+1)*4096] - a pure HOST reshape, no
replication. The 4 targets of a video become 4 free-dim segments:

    PE   : NOTHING. No matmuls, no ldweights, no PSUM.
    ACT  : per (quarter-chunk c, target t): ab = Abs(so2_seg + bias_t)
           where bias_t = -ts[(v,t)] is a [128,1] f32 column (exact).
           2 funnels per chunk (a/b sides), reading resident SBUF bf16.
    DVE  : ONE fused 2048-wide scalar_tensor_tensor per chunk:
           (iou > 0.5) * ab with free row-sum accum (iou read twice via a
           stride-0 broadcast AP). iou is host-permuted to the same
           (v, j) x (t, u) layout.
    counts (for mask.sum() + the per-target validity check): per (t, half)
           2048-wide pair ops - t in {0,1} on ACT (Sign(iou-0.5)+accum,
           exact thanks to the host bf16 nudge), t in {2,3} on DVE
           (tensor_scalar is_gt+add accum) - interleaved with the chunk
           stream so both engines stay balanced.

Raw accumulators ([128, 24] f32) are DMA'd out; host does all final
reductions in f64, decodes Sign counts, rebuilds per-target counts and
divides. bf16 storage halves DMA (6.3MB/core); accumulation stays f32.
Requires uniform num_targets (4 per video, as the harness generates);
any other shape falls back to the exact host path.
"""

import os

import numpy as np

TOPK = 3
IOU_THRESHOLD = 0.5
N_CORES = 8

# filled by kernel() on every call; test.py reads these
LAST_EXEC_TIME_NS = None
LAST_RESULTS = None

_NC_CACHE = {}

_AXON_PJRT_SO = "/opt/axon/libaxon_pjrt.so"


def _ensure_ntff_hook():
    """concourse.bass_utils hard-imports antenv.axon_hooks when tracing is
    requested (BASS_TRACE=1). Some images lack that module; provide a shim
    wired to libaxon_pjrt.so's NRT profile entry points so tracing works
    (and a missing hook degrades to an untraced run instead of crashing)."""
    try:
        from antenv.axon_hooks import get_axon_ntff_profile_hook  # noqa: F401

        return
    except ImportError:
        pass

    import contextlib
    import ctypes
    import sys
    import types

    mod = types.ModuleType("antenv.axon_hooks")
    state = {"hook": None}
    mod.set_axon_ntff_profile_hook = lambda h: state.__setitem__("hook", h)
    mod.get_axon_ntff_profile_hook = lambda: state["hook"]
    sys.modules["antenv.axon_hooks"] = mod
    try:
        import antenv

        antenv.axon_hooks = mod
    except ImportError:
        pass

    if not os.path.exists(_AXON_PJRT_SO):
        return
    lib = ctypes.CDLL(_AXON_PJRT_SO)
    if not hasattr(lib, "axon_start_nrt_profile"):
        return
    lib.axon_start_nrt_profile.argtypes = [
        ctypes.POINTER(ctypes.c_int64),
        ctypes.c_size_t,
    ]
    lib.axon_start_nrt_profile.restype = ctypes.c_int64
    lib.axon_stop_nrt_profile.argtypes = [ctypes.c_char_p]
    lib.axon_stop_nrt_profile.restype = ctypes.c_int64

    @contextlib.contextmanager
    def _hook(output_dir, device_ids):
        import jax

        jax.devices()
        if device_ids:
            ids = (ctypes.c_int64 * len(device_ids))(*device_ids)
            rc = lib.axon_start_nrt_profile(ids, len(device_ids))
        else:
            rc = lib.axon_start_nrt_profile(None, 0)
        if rc != 0:
            raise RuntimeError(f"axon_start_nrt_profile rc={rc}")
        try:
            yield
        finally:
            n = lib.axon_stop_nrt_profile(str(output_dir).encode())
            if n < 0:
                raise RuntimeError(f"axon_stop_nrt_profile rc={n}")

    mod.set_axon_ntff_profile_hook(_hook)


# geometry (uniform num_targets == 4 assumed; checked at runtime)
NV = 32     # videos per core (partgroup)
NT = 4      # targets per video
NQ = 4      # P-quarters per video row
QW = 4096   # quarter width
CW = 1024   # chunk width within a quarter


def _build_nc(M_loc, P):
    import concourse.bacc as bacc
    import concourse.bass as bass  # noqa: F401
    import concourse.mybir as mybir
    from concourse.tile import TileContext

    f32 = mybir.dt.float32
    bf16 = mybir.dt.bfloat16
    NCH = NQ  # chunks per quarter (QW // CW)
    assert QW * NQ == P and CW * NCH == QW and NV * NQ == M_loc

    nc = bacc.Bacc(enable_partition_id=False)
    iour = nc.declare_dram_parameter("iour", [M_loc, P], bf16, isOutput=False)
    so2r = nc.declare_dram_parameter("so2r", [M_loc, QW], bf16, isOutput=False)
    eo2r = nc.declare_dram_parameter("eo2r", [M_loc, QW], bf16, isOutput=False)
    ntgt = nc.declare_dram_parameter("ntgt", [M_loc, 2 * NT], f32, isOutput=False)
    # cols [0:16] = per-chunk masked loss accums, [16:24] = count accums
    out = nc.declare_dram_parameter("out", [M_loc, 3 * NT * NCH // 2], f32, isOutput=True)

    with TileContext(nc) as tc:
        with (
            tc.tile_pool(name="singles", bufs=1) as singles,
            tc.tile_pool(name="work", bufs=6) as work,
        ):
            # prime the ACT function LUT during DMA spin-up
            warm = singles.tile([M_loc, 1], f32)
            nc.vector.memset(warm, 0.0)
            nc.scalar.activation(
                out=warm, in_=warm, func=mybir.ActivationFunctionType.Abs
            )
            nc.scalar.activation(
                out=warm, in_=warm, func=mybir.ActivationFunctionType.Sign
            )
            neg_half = singles.tile([M_loc, 1], f32)
            nc.vector.memset(neg_half, -IOU_THRESHOLD)

            # resident inputs; sliced DMAs so chunk (c0,t0) starts asap
            so2_sb = singles.tile([M_loc, QW], bf16)
            eo2_sb = singles.tile([M_loc, QW], bf16)
            iou_sb = singles.tile([M_loc, 1, P], bf16)
            ntgt_sb = singles.tile([M_loc, 2 * NT], f32)

            def load_c(c):
                sl = slice(c * CW, (c + 1) * CW)
                nc.sync.dma_start(out=so2_sb[:, sl], in_=so2r[:, sl])
                nc.sync.dma_start(out=eo2_sb[:, sl], in_=eo2r[:, sl])

            def load_iou(t, c):
                sl = slice(t * QW + c * CW, t * QW + (c + 1) * CW)
                nc.sync.dma_start(out=iou_sb[:, 0, sl], in_=iour[:, sl])

            load_c(0)
            nc.sync.dma_start(out=ntgt_sb, in_=ntgt[:, :])
            for t in range(NT):
                load_iou(t, 0)
            for c in range(1, NCH):
                load_c(c)
                for t in range(NT):
                    load_iou(t, c)

            acc = singles.tile([M_loc, 3 * NT * NCH // 2], f32)
            junk_dve = singles.tile([M_loc, 2, CW], bf16, tag="junk_dve")
            junk_cnt = singles.tile([M_loc, 2 * CW], bf16, tag="junk_cnt")
            junk_act = singles.tile([M_loc, 2 * CW], bf16, tag="junk_act")

            NCOL = NT * NCH  # 16 loss columns
            for c in range(NCH):
                for t in range(NT):
                    csl = slice(c * CW, (c + 1) * CW)
                    ab = work.tile([M_loc, 2, CW], bf16, tag="ab")
                    nc.scalar.activation(
                        out=ab[:, 0, :],
                        in_=so2_sb[:, csl],
                        func=mybir.ActivationFunctionType.Abs,
                        bias=ntgt_sb[:, t : t + 1],
                        scale=1.0,
                    )
                    nc.scalar.activation(
                        out=ab[:, 1, :],
                        in_=eo2_sb[:, csl],
                        func=mybir.ActivationFunctionType.Abs,
                        bias=ntgt_sb[:, NT + t : NT + t + 1],
                        scale=1.0,
                    )
                    k = c * NT + t
                    isl = slice(t * QW + c * CW, t * QW + (c + 1) * CW)
                    nc.vector.scalar_tensor_tensor(
                        out=junk_dve[:, :, :],
                        in0=iou_sb[:, :, isl].broadcast_to([M_loc, 2, CW]),
                        scalar=IOU_THRESHOLD,
                        in1=ab[:, :, :],
                        op0=mybir.AluOpType.is_gt,
                        op1=mybir.AluOpType.mult,
                        accum_out=acc[:, k : k + 1],
                    )
                    # counts: one 2048-wide pair op per (t, half); t 0/1 on
                    # ACT (Sign), t 2/3 on DVE - spread through the stream
                    if c in (1, 3):
                        half = c // 2
                        kc = NCOL + half * NT + t
                        psl = slice(
                            t * QW + half * 2 * CW, t * QW + (half + 1) * 2 * CW
                        )
                        if t < 2:
                            nc.scalar.activation(
                                out=junk_act[:, :],
                                in_=iou_sb[:, 0, psl],
                                func=mybir.ActivationFunctionType.Sign,
                                bias=neg_half[:, 0:1],
                                scale=1.0,
                                accum_out=acc[:, kc : kc + 1],
                            )
                        else:
                            nc.vector.tensor_scalar(
                                out=junk_cnt[:, :],
                                in0=iou_sb[:, 0, psl],
                                scalar1=IOU_THRESHOLD,
                                scalar2=None,
                                op0=mybir.AluOpType.is_gt,
                                op1=mybir.AluOpType.add,
                                accum_out=acc[:, kc : kc + 1],
                            )

            nc.sync.dma_start(out=out[:, :], in_=acc)

    nc.compile()
    return nc


def _scatter_m2s(num_targets, S, M):
    """target index -> source video index, mirroring jnp.repeat(
    arange(S), num_targets, total_repeat_length=M)."""
    cum = np.cumsum(num_targets.astype(np.int64))
    idx = np.searchsorted(cum, np.arange(M), side="right")
    return np.clip(idx, 0, S - 1).astype(np.int64)


def _numpy_reference(start_offset, end_offset, tgt_moments, num_targets, iou2ds, mask2d):
    """Exact numpy replica of reference.py (fallback path)."""
    M, N, _ = iou2ds.shape
    S, P = start_offset.shape
    scatter = _scatter_m2s(num_targets, S, M)
    so = start_offset[scatter]
    eo = end_offset[scatter]
    r, c = np.nonzero(mask2d)
    if r.shape[0] < P:
        pad = P - r.shape[0]
        r = np.concatenate([r, np.zeros(pad, dtype=r.dtype)])
        c = np.concatenate([c, np.zeros(pad, dtype=c.dtype)])
    else:
        r, c = r[:P], c[:P]
    iou1 = iou2ds.reshape(M, N * N)[:, r * N + c]
    topk_idx = np.argsort(-iou1, axis=1, kind="stable")[:, :TOPK]
    mask = np.zeros((M, P), dtype=np.float32)
    np.put_along_axis(mask, topk_idx, 1.0, axis=1)
    mask = np.where(iou1 > IOU_THRESHOLD, np.float32(1.0), mask)
    starts = (r.astype(np.float32) / N)[None, :]
    ends = ((c.astype(np.float32) + 1.0) / N)[None, :]
    sot = tgt_moments[:, 0:1] - starts
    eot = tgt_moments[:, 1:2] - ends
    loss = np.abs(so - sot) + np.abs(eo - eot)
    return np.float32((loss * mask).sum(dtype=np.float64) / mask.sum(dtype=np.float64))


def kernel(**inputs):
    global LAST_EXEC_TIME_NS, LAST_RESULTS
    _ensure_ntff_hook()
    import ml_dtypes

    from concourse.bass_utils import run_bass_kernel_spmd

    start_offset = np.asarray(inputs["start_offset"], dtype=np.float32)
    end_offset = np.asarray(inputs["end_offset"], dtype=np.float32)
    tgt_moments = np.asarray(inputs["tgt_moments"], dtype=np.float32)
    num_targets = np.asarray(inputs["num_targets"])
    iou2ds = np.asarray(inputs["iou2ds"], dtype=np.float32)
    mask2d = np.asarray(inputs["mask2d"])

    bf16 = ml_dtypes.bfloat16

    M, N, _ = iou2ds.shape
    S, P = start_offset.shape
    M_loc = M // N_CORES

    # the video-major layout needs the rigid uniform structure the harness
    # generates; otherwise use the exact host path
    scatter = _scatter_m2s(num_targets, S, M)
    uniform = (
        M == N_CORES * NV * NT
        and S == N_CORES * NV
        and P == NQ * QW
        and M_loc == NV * NQ
        and (scatter == np.repeat(np.arange(S), NT)).all()
    )
    if not uniform:
        return _numpy_reference(
            start_offset, end_offset, tgt_moments, num_targets, iou2ds, mask2d
        )

    # proposal-grid constants from mask2d (row-major nonzero, padded like jnp)
    r, c = np.nonzero(mask2d)
    if r.shape[0] < P:
        pad = P - r.shape[0]
        r = np.concatenate([r, np.zeros(pad, dtype=r.dtype)])
        c = np.concatenate([c, np.zeros(pad, dtype=c.dtype)])
    else:
        r, c = r[:P], c[:P]
    starts = r.astype(np.float32) / np.float32(N)
    ends = (c.astype(np.float32) + np.float32(1.0)) / np.float32(N)

    # iou1ds = iou2ds[:, r, c]; identity reshape when mask2d is all ones
    flat_idx = r.astype(np.int64) * N + c.astype(np.int64)
    iou_flat = iou2ds.reshape(M, N * N)
    if not (flat_idx == np.arange(P)).all():
        iou_flat = np.ascontiguousarray(iou_flat[:, flat_idx])
    # bf16 halves the iou DMA bytes; nudge values that round onto the 0.5
    # threshold one bf16 ulp away toward their f32 value so the device
    # compare matches f32 exactly AND Sign(iou-0.5) is strictly +-1
    iou_bf16 = iou_flat.astype(bf16)
    on_thr = iou_bf16 == bf16(IOU_THRESHOLD)
    above = on_thr & (iou_flat > np.float32(IOU_THRESHOLD))
    below = on_thr & ~above
    if above.any():
        iou_bf16[above] = bf16(0.50390625)  # nextafter(0.5, up) in bf16
    if below.any():
        iou_bf16[below] = bf16(0.498046875)  # nextafter(0.5, down) in bf16

    # fold grid constants into the offsets: loss_a = |so2 - ts|
    so2_full = (start_offset + starts[None, :]).astype(bf16)
    eo2_full = (end_offset + ends[None, :]).astype(bf16)

    in_maps = []
    for core in range(N_CORES):
        # video-major reshape: partition q = v*NQ + j holds quarter j of
        # video v; iou segment t of partition q = target (v, t), quarter j
        so_c = so2_full[core * NV : (core + 1) * NV]  # [NV, P]
        eo_c = eo2_full[core * NV : (core + 1) * NV]
        so2r = np.ascontiguousarray(so_c.reshape(NV * NQ, QW))
        eo2r = np.ascontiguousarray(eo_c.reshape(NV * NQ, QW))
        iou_c = iou_bf16[core * M_loc : (core + 1) * M_loc]  # [NV*NT, P]
        iour = np.ascontiguousarray(
            iou_c.reshape(NV, NT, NQ, QW).transpose(0, 2, 1, 3).reshape(M_loc, P)
        )
        tg = tgt_moments[core * M_loc : (core + 1) * M_loc]  # [NV*NT, 2]
        tg_v = tg.reshape(NV, NT, 2)
        ntgt = np.empty((M_loc, 2 * NT), dtype=np.float32)
        # per partition (v, j): cols [0:NT] = -ts of targets 0..3 of video v,
        # cols [NT:2NT] = -te (same for all j)
        ntgt[:, :NT] = np.repeat(-tg_v[:, :, 0], NQ, axis=0)
        ntgt[:, NT:] = np.repeat(-tg_v[:, :, 1], NQ, axis=0)

        in_maps.append(
            {"iour": iour, "so2r": so2r, "eo2r": eo2r, "ntgt": ntgt}
        )

    cache_key = (M_loc, P, "video-major")
    if cache_key not in _NC_CACHE:
        _NC_CACHE[cache_key] = _build_nc(M_loc, P)
    nc = _NC_CACHE[cache_key]

    res = run_bass_kernel_spmd(nc, in_maps, list(range(N_CORES)))
    LAST_EXEC_TIME_NS = res.exec_time_ns
    LAST_RESULTS = res

    NCOL = NT * NQ  # 16 loss cols
    loss_sum = 0.0
    mask_sum = 0.0
    min_count = np.inf
    for core in range(N_CORES):
        part = res.results[core]["out"].astype(np.float64)  # [M_loc, 24]
        loss_sum += part[:, :NCOL].sum()
        cnt = part[:, NCOL:]  # [M_loc, 2*NT]: col half*NT+t
        # ACT Sign columns (t < 2 in each half): count = (sum_sign + 2CW)/2
        for half in range(2):
            for t in range(2):
                kc = half * NT + t
                cnt[:, kc] = (cnt[:, kc] + 2.0 * CW) * 0.5
        mask_sum += cnt.sum()
        # per-target counts: target (v, t) = sum over quarters j and halves
        cnt_v = cnt.reshape(NV, NQ, 2, NT)  # (v, j, half, t)
        per_target = cnt_v.sum(axis=(1, 2))  # [NV, NT]
        min_count = min(min_count, per_target.min())

    if min_count < TOPK:
        # some target's top-k reaches below the threshold: the threshold
        # mask is not exact there -> use the exact (slow) host path
        return _numpy_reference(
            start_offset, end_offset, tgt_moments, num_targets, iou2ds, mask2d
        )

    return np.float32(loss_sum / mask_sum)


# revision 28
# speedup vs baseline: 1.1450x; 1.1450x over previous
"""Trainium2 Bass kernel for nn_BboxRegressionLoss (topk_masking).

Math notes
----------
reference computes, with iou1ds = iou2ds reshaped [M, P] (mask2d all-ones):
    mask = scatter(top3_idx) | (iou1ds > 0.5)
    loss = |so + starts - tgt_s| + |eo + ends - tgt_e|     (per [M, P] element)
    out  = (loss * mask).sum() / mask.sum()

Key identity: if a row has >= TOPK elements with iou > 0.5, its top-TOPK
elements are all already inside the threshold mask, so mask == (iou > 0.5)
EXACTLY for that row. Per-row (per-target) counts are computed on device
(needed for mask.sum() anyway); host verifies the identity for every target
and falls back to an exact numpy replica otherwise.

VIDEO-MAJOR layout (v7) - eliminates the PE entirely
----------------------------------------------------
Previous structure replicated the K=32 so/eo source rows to 128 target
partitions with matmuls (PE measured ~47us busy - as hot as ACT/DVE).
Instead, partitions are now (video v, P-quarter j): 32 videos x 4 quarters.
Each partition holds so2[v, j*4096:(j+1)*4096] - a pure HOST reshape, no
replication. The 4 targets of a video become 4 free-dim segments:

    PE   : NOTHING. No matmuls, no ldweights, no PSUM.
    ACT  : per (quarter-chunk c, target t): ab = Abs(so2_seg + bias_t)
           where bias_t = -ts[(v,t)] is a [128,1] f32 column (exact).
           2 funnels per chunk (a/b sides), reading resident SBUF bf16.
    DVE  : ONE fused 2048-wide scalar_tensor_tensor per chunk:
           (iou > 0.5) * ab with free row-sum accum (iou read twice via a
           stride-0 broadcast AP). iou is host-permuted to the same
           (v, j) x (t, u) layout.
    counts (for mask.sum() + the per-target validity check): per (t, half)
           2048-wide pair ops - t in {0,1} on ACT (Sign(iou-0.5)+accum,
           exact thanks to the host bf16 nudge), t in {2,3} on DVE
           (tensor_scalar is_gt+add accum) - interleaved with the chunk
           stream so both engines stay balanced.

Raw accumulators ([128, 24] f32) are DMA'd out; host does all final
reductions in f64, decodes Sign counts, rebuilds per-target counts and
divides. bf16 storage halves DMA (6.3MB/core); accumulation stays f32.
Requires uniform num_targets (4 per video, as the harness generates);
any other shape falls back to the exact host path.
"""

import os

import numpy as np

TOPK = 3
IOU_THRESHOLD = 0.5
N_CORES = 8

# filled by kernel() on every call; test.py reads these
LAST_EXEC_TIME_NS = None
LAST_RESULTS = None

_NC_CACHE = {}

_AXON_PJRT_SO = "/opt/axon/libaxon_pjrt.so"


def _ensure_ntff_hook():
    """concourse.bass_utils hard-imports antenv.axon_hooks when tracing is
    requested (BASS_TRACE=1). Some images lack that module; provide a shim
    wired to libaxon_pjrt.so's NRT profile entry points so tracing works
    (and a missing hook degrades to an untraced run instead of crashing)."""
    try:
        from antenv.axon_hooks import get_axon_ntff_profile_hook  # noqa: F401

        return
    except ImportError:
        pass

    import contextlib
    import ctypes
    import sys
    import types

    mod = types.ModuleType("antenv.axon_hooks")
    state = {"hook": None}
    mod.set_axon_ntff_profile_hook = lambda h: state.__setitem__("hook", h)
    mod.get_axon_ntff_profile_hook = lambda: state["hook"]
    sys.modules["antenv.axon_hooks"] = mod
    try:
        import antenv

        antenv.axon_hooks = mod
    except ImportError:
        pass

    if not os.path.exists(_AXON_PJRT_SO):
        return
    lib = ctypes.CDLL(_AXON_PJRT_SO)
    if not hasattr(lib, "axon_start_nrt_profile"):
        return
    lib.axon_start_nrt_profile.argtypes = [
        ctypes.POINTER(ctypes.c_int64),
        ctypes.c_size_t,
    ]
    lib.axon_start_nrt_profile.restype = ctypes.c_int64
    lib.axon_stop_nrt_profile.argtypes = [ctypes.c_char_p]
    lib.axon_stop_nrt_profile.restype = ctypes.c_int64

    @contextlib.contextmanager
    def _hook(output_dir, device_ids):
        import jax

        jax.devices()
        if device_ids:
            ids = (ctypes.c_int64 * len(device_ids))(*device_ids)
            rc = lib.axon_start_nrt_profile(ids, len(device_ids))
        else:
            rc = lib.axon_start_nrt_profile(None, 0)
        if rc != 0:
            raise RuntimeError(f"axon_start_nrt_profile rc={rc}")
        try:
            yield
        finally:
            n = lib.axon_stop_nrt_profile(str(output_dir).encode())
            if n < 0:
                raise RuntimeError(f"axon_stop_nrt_profile rc={n}")

    mod.set_axon_ntff_profile_hook(_hook)


# geometry (uniform num_targets == 4 assumed; checked at runtime)
NV = 32     # videos per core (partgroup)
NT = 4      # targets per video
NQ = 4      # P-quarters per video row
QW = 4096   # quarter width
CW = 1024   # chunk width within a quarter


def _build_nc(M_loc, P):
    import concourse.bacc as bacc
    import concourse.bass as bass  # noqa: F401
    import concourse.mybir as mybir
    from concourse.tile import TileContext

    f32 = mybir.dt.float32
    bf16 = mybir.dt.bfloat16
    NCH = NQ  # chunks per quarter (QW // CW)
    assert QW * NQ == P and CW * NCH == QW and NV * NQ == M_loc

    nc = bacc.Bacc(enable_partition_id=False)
    iour = nc.declare_dram_parameter("iour", [M_loc, P], bf16, isOutput=False)
    so2r = nc.declare_dram_parameter("so2r", [M_loc, QW], bf16, isOutput=False)
    eo2r = nc.declare_dram_parameter("eo2r", [M_loc, QW], bf16, isOutput=False)
    ntgt = nc.declare_dram_parameter("ntgt", [M_loc, 2 * NT], f32, isOutput=False)
    # cols [0:8] = per-chunk masked loss accums, [8:16] = count accums
    out = nc.declare_dram_parameter("out", [M_loc, 4 * NT], f32, isOutput=True)

    with TileContext(nc) as tc:
        with (
            tc.tile_pool(name="singles", bufs=1) as singles,
            tc.tile_pool(name="work", bufs=6) as work,
        ):
            # prime the ACT function LUT during DMA spin-up
            warm = singles.tile([M_loc, 1], f32)
            nc.vector.memset(warm, 0.0)
            nc.scalar.activation(
                out=warm, in_=warm, func=mybir.ActivationFunctionType.Abs
            )
            nc.scalar.activation(
                out=warm, in_=warm, func=mybir.ActivationFunctionType.Sign
            )
            neg_half = singles.tile([M_loc, 1], f32)
            nc.vector.memset(neg_half, -IOU_THRESHOLD)

            # resident inputs; sliced DMAs so chunk (c0,t0) starts asap
            so2_sb = singles.tile([M_loc, QW], bf16)
            eo2_sb = singles.tile([M_loc, QW], bf16)
            iou_sb = singles.tile([M_loc, 1, P], bf16)
            ntgt_sb = singles.tile([M_loc, 2 * NT], f32)

            W = 2 * CW  # 2048-wide ops: half a quarter per chunk

            def load_c(c2):
                sl = slice(c2 * W, (c2 + 1) * W)
                nc.sync.dma_start(out=so2_sb[:, sl], in_=so2r[:, sl])
                nc.sync.dma_start(out=eo2_sb[:, sl], in_=eo2r[:, sl])

            def load_iou(t, c2):
                sl = slice(t * QW + c2 * W, t * QW + (c2 + 1) * W)
                nc.sync.dma_start(out=iou_sb[:, 0, sl], in_=iour[:, sl])

            load_c(0)
            nc.sync.dma_start(out=ntgt_sb, in_=ntgt[:, :])
            for t in range(NT):
                load_iou(t, 0)
            load_c(1)
            for t in range(NT):
                load_iou(t, 1)

            NCOL = 2 * NT  # 8 loss columns (c2, t)
            acc = singles.tile([M_loc, 2 * NCOL], f32)
            junk_dve = singles.tile([M_loc, 2, W], bf16, tag="junk_dve")
            junk_cnt = singles.tile([M_loc, W], bf16, tag="junk_cnt")
            junk_act = singles.tile([M_loc, W], bf16, tag="junk_act")

            for c2 in range(2):
                for t in range(NT):
                    csl = slice(c2 * W, (c2 + 1) * W)
                    ab = work.tile([M_loc, 2, W], bf16, tag="ab")
                    nc.scalar.activation(
                        out=ab[:, 0, :],
                        in_=so2_sb[:, csl],
                        func=mybir.ActivationFunctionType.Abs,
                        bias=ntgt_sb[:, t : t + 1],
                        scale=1.0,
                    )
                    nc.scalar.activation(
                        out=ab[:, 1, :],
                        in_=eo2_sb[:, csl],
                        func=mybir.ActivationFunctionType.Abs,
                        bias=ntgt_sb[:, NT + t : NT + t + 1],
                        scale=1.0,
                    )
                    k = c2 * NT + t
                    isl = slice(t * QW + c2 * W, t * QW + (c2 + 1) * W)
                    nc.vector.scalar_tensor_tensor(
                        out=junk_dve[:, :, :],
                        in0=iou_sb[:, :, isl].broadcast_to([M_loc, 2, W]),
                        scalar=IOU_THRESHOLD,
                        in1=ab[:, :, :],
                        op0=mybir.AluOpType.is_gt,
                        op1=mybir.AluOpType.mult,
                        accum_out=acc[:, k : k + 1],
                    )
                    # counts: same 2048-slice as the chunk; t 0/1 on ACT
                    # (Sign), t 2/3 on DVE - spread through the stream
                    kc = NCOL + k
                    if t < 2:
                        nc.scalar.activation(
                            out=junk_act[:, :],
                            in_=iou_sb[:, 0, isl],
                            func=mybir.ActivationFunctionType.Sign,
                            bias=neg_half[:, 0:1],
                            scale=1.0,
                            accum_out=acc[:, kc : kc + 1],
                        )
                    else:
                        nc.vector.tensor_scalar(
                            out=junk_cnt[:, :],
                            in0=iou_sb[:, 0, isl],
                            scalar1=IOU_THRESHOLD,
                            scalar2=None,
                            op0=mybir.AluOpType.is_gt,
                            op1=mybir.AluOpType.add,
                            accum_out=acc[:, kc : kc + 1],
                        )

            nc.sync.dma_start(out=out[:, :], in_=acc)

    nc.compile()
    return nc


def _scatter_m2s(num_targets, S, M):
    """target index -> source video index, mirroring jnp.repeat(
    arange(S), num_targets, total_repeat_length=M)."""
    cum = np.cumsum(num_targets.astype(np.int64))
    idx = np.searchsorted(cum, np.arange(M), side="right")
    return np.clip(idx, 0, S - 1).astype(np.int64)


def _numpy_reference(start_offset, end_offset, tgt_moments, num_targets, iou2ds, mask2d):
    """Exact numpy replica of reference.py (fallback path)."""
    M, N, _ = iou2ds.shape
    S, P = start_offset.shape
    scatter = _scatter_m2s(num_targets, S, M)
    so = start_offset[scatter]
    eo = end_offset[scatter]
    r, c = np.nonzero(mask2d)
    if r.shape[0] < P:
        pad = P - r.shape[0]
        r = np.concatenate([r, np.zeros(pad, dtype=r.dtype)])
        c = np.concatenate([c, np.zeros(pad, dtype=c.dtype)])
    else:
        r, c = r[:P], c[:P]
    iou1 = iou2ds.reshape(M, N * N)[:, r * N + c]
    topk_idx = np.argsort(-iou1, axis=1, kind="stable")[:, :TOPK]
    mask = np.zeros((M, P), dtype=np.float32)
    np.put_along_axis(mask, topk_idx, 1.0, axis=1)
    mask = np.where(iou1 > IOU_THRESHOLD, np.float32(1.0), mask)
    starts = (r.astype(np.float32) / N)[None, :]
    ends = ((c.astype(np.float32) + 1.0) / N)[None, :]
    sot = tgt_moments[:, 0:1] - starts
    eot = tgt_moments[:, 1:2] - ends
    loss = np.abs(so - sot) + np.abs(eo - eot)
    return np.float32((loss * mask).sum(dtype=np.float64) / mask.sum(dtype=np.float64))


def kernel(**inputs):
    global LAST_EXEC_TIME_NS, LAST_RESULTS
    _ensure_ntff_hook()
    import ml_dtypes

    from concourse.bass_utils import run_bass_kernel_spmd

    start_offset = np.asarray(inputs["start_offset"], dtype=np.float32)
    end_offset = np.asarray(inputs["end_offset"], dtype=np.float32)
    tgt_moments = np.asarray(inputs["tgt_moments"], dtype=np.float32)
    num_targets = np.asarray(inputs["num_targets"])
    iou2ds = np.asarray(inputs["iou2ds"], dtype=np.float32)
    mask2d = np.asarray(inputs["mask2d"])

    bf16 = ml_dtypes.bfloat16

    M, N, _ = iou2ds.shape
    S, P = start_offset.shape
    M_loc = M // N_CORES

    # the video-major layout needs the rigid uniform structure the harness
    # generates; otherwise use the exact host path
    scatter = _scatter_m2s(num_targets, S, M)
    uniform = (
        M == N_CORES * NV * NT
        and S == N_CORES * NV
        and P == NQ * QW
        and M_loc == NV * NQ
        and (scatter == np.repeat(np.arange(S), NT)).all()
    )
    if not uniform:
        return _numpy_reference(
            start_offset, end_offset, tgt_moments, num_targets, iou2ds, mask2d
        )

    # proposal-grid constants from mask2d (row-major nonzero, padded like jnp)
    r, c = np.nonzero(mask2d)
    if r.shape[0] < P:
        pad = P - r.shape[0]
        r = np.concatenate([r, np.zeros(pad, dtype=r.dtype)])
        c = np.concatenate([c, np.zeros(pad, dtype=c.dtype)])
    else:
        r, c = r[:P], c[:P]
    starts = r.astype(np.float32) / np.float32(N)
    ends = (c.astype(np.float32) + np.float32(1.0)) / np.float32(N)

    # iou1ds = iou2ds[:, r, c]; identity reshape when mask2d is all ones
    flat_idx = r.astype(np.int64) * N + c.astype(np.int64)
    iou_flat = iou2ds.reshape(M, N * N)
    if not (flat_idx == np.arange(P)).all():
        iou_flat = np.ascontiguousarray(iou_flat[:, flat_idx])
    # bf16 halves the iou DMA bytes; nudge values that round onto the 0.5
    # threshold one bf16 ulp away toward their f32 value so the device
    # compare matches f32 exactly AND Sign(iou-0.5) is strictly +-1
    iou_bf16 = iou_flat.astype(bf16)
    on_thr = iou_bf16 == bf16(IOU_THRESHOLD)
    above = on_thr & (iou_flat > np.float32(IOU_THRESHOLD))
    below = on_thr & ~above
    if above.any():
        iou_bf16[above] = bf16(0.50390625)  # nextafter(0.5, up) in bf16
    if below.any():
        iou_bf16[below] = bf16(0.498046875)  # nextafter(0.5, down) in bf16

    # fold grid constants into the offsets: loss_a = |so2 - ts|
    so2_full = (start_offset + starts[None, :]).astype(bf16)
    eo2_full = (end_offset + ends[None, :]).astype(bf16)

    in_maps = []
    for core in range(N_CORES):
        # video-major reshape: partition q = v*NQ + j holds quarter j of
        # video v; iou segment t of partition q = target (v, t), quarter j
        so_c = so2_full[core * NV : (core + 1) * NV]  # [NV, P]
        eo_c = eo2_full[core * NV : (core + 1) * NV]
        so2r = np.ascontiguousarray(so_c.reshape(NV * NQ, QW))
        eo2r = np.ascontiguousarray(eo_c.reshape(NV * NQ, QW))
        iou_c = iou_bf16[core * M_loc : (core + 1) * M_loc]  # [NV*NT, P]
        iour = np.ascontiguousarray(
            iou_c.reshape(NV, NT, NQ, QW).transpose(0, 2, 1, 3).reshape(M_loc, P)
        )
        tg = tgt_moments[core * M_loc : (core + 1) * M_loc]  # [NV*NT, 2]
        tg_v = tg.reshape(NV, NT, 2)
        ntgt = np.empty((M_loc, 2 * NT), dtype=np.float32)
        # per partition (v, j): cols [0:NT] = -ts of targets 0..3 of video v,
        # cols [NT:2NT] = -te (same for all j)
        ntgt[:, :NT] = np.repeat(-tg_v[:, :, 0], NQ, axis=0)
        ntgt[:, NT:] = np.repeat(-tg_v[:, :, 1], NQ, axis=0)

        in_maps.append(
            {"iour": iour, "so2r": so2r, "eo2r": eo2r, "ntgt": ntgt}
        )

    cache_key = (M_loc, P, "video-major")
    if cache_key not in _NC_CACHE:
        _NC_CACHE[cache_key] = _build_nc(M_loc, P)
    nc = _NC_CACHE[cache_key]

    res = run_bass_kernel_spmd(nc, in_maps, list(range(N_CORES)))
    LAST_EXEC_TIME_NS = res.exec_time_ns
    LAST_RESULTS = res

    NCOL = 2 * NT  # 8 loss cols (c2, t)
    loss_sum = 0.0
    mask_sum = 0.0
    min_count = np.inf
    for core in range(N_CORES):
        part = res.results[core]["out"].astype(np.float64)  # [M_loc, 16]
        loss_sum += part[:, :NCOL].sum()
        cnt = part[:, NCOL:]  # [M_loc, 2*NT]: col c2*NT+t
        # ACT Sign columns (t < 2 in each c2-half): count = (sum_sign+2048)/2
        for c2 in range(2):
            for t in range(2):
                kc = c2 * NT + t
                cnt[:, kc] = (cnt[:, kc] + 2.0 * CW) * 0.5
        mask_sum += cnt.sum()
        # per-target counts: target (v, t) = sum over quarters j and halves
        cnt_v = cnt.reshape(NV, NQ, 2, NT)  # (v, j, c2, t)
        per_target = cnt_v.sum(axis=(1, 2))  # [NV, NT]
        min_count = min(min_count, per_target.min())

    if min_count < TOPK:
        # some target's top-k reaches below the threshold: the threshold
        # mask is not exact there -> use the exact (slow) host path
        return _numpy_reference(
            start_offset, end_offset, tgt_moments, num_targets, iou2ds, mask2d
        )

    return np.float32(loss_sum / mask_sum)


# revision 29
# speedup vs baseline: 1.1978x; 1.0462x over previous
"""Trainium2 Bass kernel for nn_BboxRegressionLoss (topk_masking).

Math notes
----------
reference computes, with iou1ds = iou2ds reshaped [M, P] (mask2d all-ones):
    mask = scatter(top3_idx) | (iou1ds > 0.5)
    loss = |so + starts - tgt_s| + |eo + ends - tgt_e|     (per [M, P] element)
    out  = (loss * mask).sum() / mask.sum()

Key identity: if a row has >= TOPK elements with iou > 0.5, its top-TOPK
elements are all already inside the threshold mask, so mask == (iou > 0.5)
EXACTLY for that row. We compute per-row counts of (iou > 0.5) on device
anyway (needed for mask.sum()), so we can verify the identity for every row
after the fact and fall back to a full numpy replica in the (practically
impossible for uniform iou) case where some row has fewer than TOPK
above-threshold elements.

Device layout (per core, M_loc = 128 targets on partitions, P chunked):
    PE     : replicate K source-offset rows -> 128 target partitions via a
             0/1 matmul (avoids re-reading so/eo 4x from HBM)
    ACT    : a = Abs(so2rep - tgt_s), b = Abs(eo2rep - tgt_e)   (bias fusion)
    DVE    : scalar_tensor_tensor (iou > 0.5) * a  with fused row-sum accum
             (and same for b); mask counts via tensor_scalar accum on DVE
             for some chunks and Sign(iou-0.5) accum on ACT for the rest
             (DVE/ACT load balancing; both are exact thanks to the host-side
             threshold nudge that moves bf16 values off 0.5)
Host folds the `starts`/`ends` proposal-grid constants into so/eo (so2/eo2),
sums the 8 x [128, 2] partials in f64 and divides.

bf16 storage halves the DMA bytes; accumulation stays f32. Measured
end-to-end rel err vs the f32 reference is ~7e-6. Measured HW exec time
64-65us on 8 cores (6.3MB HBM reads per core; ~19us of that is fixed
kernel entry/exit barrier+drain overhead; DVE/ACT both run gap-free at
~47-50us busy, the compute-pass floor for this op structure).

Restructures tried and measured AT or ABOVE this baseline (do not repeat):
- 2048-wide single-PSUM-tile chunks (one wide Abs + one wide broadcast stt):
  engine busy drops ~8us but the coarse PSUM WAR chain serializes PE and
  measured 81us. Fine-granularity PSUM + wide stt + pair-width counts +
  raw-accum DMA-out measured 65.0-66.6us = tied with this baseline.
- abs_max as stt op1 (single-pass masked-abs from PSUM, no ACT funnel):
  REJECTED by the walrus ISA check (NCC_IXCG864); is_le+max passes but only
  gives relu (loses the negative half), so the ACT Abs funnel is mandatory.
- DVE fast modes: plain tensor_scalar is ~4x (0.4ns/elem) and tensor_tensor
  ~2x, but EVERY reducing variant (accum_out / tensor_reduce / stt) runs 1x,
  so tt-2x mask-mult + separate sum always loses to the fused 1x stt.
- PE p-state: matmuls measure ~1.2ns/row (mid clock); warm-up matmuls do
  not improve it. PE-side count via ones-matmul is therefore too slow, and
  PSUM's 8 banks leave no room for a count accumulator anyway.
- Counts: ACT-Sign vs DVE-tensor_scalar split near 50/50 is the LP optimum;
  shifting 6/2 toward DVE measured +4us. Run-to-run variance is +-1.5us in
  the fast device state, with occasional ~+15% slow-clock sessions.
"""

import os

import numpy as np

TOPK = 3
IOU_THRESHOLD = 0.5
N_CORES = 8

# filled by kernel() on every call; test.py reads these
LAST_EXEC_TIME_NS = None
LAST_RESULTS = None

_NC_CACHE = {}

_AXON_PJRT_SO = "/opt/axon/libaxon_pjrt.so"


def _ensure_ntff_hook():
    """concourse.bass_utils hard-imports antenv.axon_hooks when tracing is
    requested (BASS_TRACE=1). Some images lack that module; provide a shim
    wired to libaxon_pjrt.so's NRT profile entry points so tracing works
    (and a missing hook degrades to an untraced run instead of crashing)."""
    try:
        from antenv.axon_hooks import get_axon_ntff_profile_hook  # noqa: F401

        return
    except ImportError:
        pass

    import contextlib
    import ctypes
    import sys
    import types

    mod = types.ModuleType("antenv.axon_hooks")
    state = {"hook": None}
    mod.set_axon_ntff_profile_hook = lambda h: state.__setitem__("hook", h)
    mod.get_axon_ntff_profile_hook = lambda: state["hook"]
    sys.modules["antenv.axon_hooks"] = mod
    try:
        import antenv

        antenv.axon_hooks = mod
    except ImportError:
        pass

    if not os.path.exists(_AXON_PJRT_SO):
        return
    lib = ctypes.CDLL(_AXON_PJRT_SO)
    if not hasattr(lib, "axon_start_nrt_profile"):
        return
    lib.axon_start_nrt_profile.argtypes = [
        ctypes.POINTER(ctypes.c_int64),
        ctypes.c_size_t,
    ]
    lib.axon_start_nrt_profile.restype = ctypes.c_int64
    lib.axon_stop_nrt_profile.argtypes = [ctypes.c_char_p]
    lib.axon_stop_nrt_profile.restype = ctypes.c_int64

    @contextlib.contextmanager
    def _hook(output_dir, device_ids):
        import jax

        jax.devices()
        if device_ids:
            ids = (ctypes.c_int64 * len(device_ids))(*device_ids)
            rc = lib.axon_start_nrt_profile(ids, len(device_ids))
        else:
            rc = lib.axon_start_nrt_profile(None, 0)
        if rc != 0:
            raise RuntimeError(f"axon_start_nrt_profile rc={rc}")
        try:
            yield
        finally:
            n = lib.axon_stop_nrt_profile(str(output_dir).encode())
            if n < 0:
                raise RuntimeError(f"axon_stop_nrt_profile rc={n}")

    mod.set_axon_ntff_profile_hook(_hook)


def _build_nc(K, M_loc, P, C):
    import concourse.bacc as bacc
    import concourse.bass as bass
    import concourse.mybir as mybir
    from concourse.tile import TileContext

    f32 = mybir.dt.float32
    bf16 = mybir.dt.bfloat16
    NCH = P // C
    assert P % C == 0 and C % 512 == 0
    MMW = C // 512  # matmuls per chunk per tensor (PSUM bank = 512 f32)

    nc = bacc.Bacc(enable_partition_id=False)
    iou = nc.declare_dram_parameter("iou", [M_loc, P], bf16, isOutput=False)
    so2 = nc.declare_dram_parameter("so2", [K, P], bf16, isOutput=False)
    eo2 = nc.declare_dram_parameter("eo2", [K, P], bf16, isOutput=False)
    repl = nc.declare_dram_parameter("repl", [K, M_loc], bf16, isOutput=False)
    ntgt = nc.declare_dram_parameter("ntgt", [M_loc, 2], f32, isOutput=False)
    out = nc.declare_dram_parameter("out", [M_loc, 2], f32, isOutput=True)

    with TileContext(nc) as tc:
        with (
            tc.tile_pool(name="singles", bufs=1) as singles,
            # one slot per chunk: iou DMAs are all emitted up-front, so slots
            # must never be recycled (recycling would need WAR deps on readers
            # that don't exist yet at emission time)
            tc.tile_pool(name="io", bufs=P // C) as io,
            tc.tile_pool(name="work", bufs=6) as work,
            tc.tile_pool(name="psum", bufs=2, space="PSUM") as psum,
        ):
            # prime the ACT function LUT during DMA spin-up: the first
            # activation triggers a ~1.3us ACT_TABLE_LOAD; run a dummy op
            # with no DMA dependency so it happens at t~0 instead of
            # delaying chunk 0
            warm = singles.tile([M_loc, 1], f32)
            nc.vector.memset(warm, 0.0)
            nc.scalar.activation(
                out=warm, in_=warm, func=mybir.ActivationFunctionType.Abs
            )
            nc.scalar.activation(
                out=warm, in_=warm, func=mybir.ActivationFunctionType.Sign
            )

            R_sb = singles.tile([K, M_loc], bf16)
            nc.sync.dma_start(out=R_sb, in_=repl[:, :])
            ntgt_sb = singles.tile([M_loc, 2], f32)
            nc.sync.dma_start(out=ntgt_sb, in_=ntgt[:, :])
            # source-offset rows stay resident (bf16 [K, P] = K partitions x 32KB).
            # Loaded as one tile PER CHUNK-GROUP so early matmuls don't wait on
            # the whole 1MB transfer (Tile deps are per-tile). DMA emission
            # order: piece 0 + the first iou chunks FIRST so the pipeline
            # fills immediately, remaining pieces next, rest of iou after.
            so_piece = C
            so2_sbs, eo2_sbs, iou_tiles = [], [], []

            def load_piece(pi):
                psl = slice(pi * so_piece, (pi + 1) * so_piece)
                s_t = singles.tile([K, so_piece], bf16, tag=f"so2_sb{pi}")
                nc.sync.dma_start(out=s_t, in_=so2[:, psl])
                so2_sbs.append(s_t)
                e_t = singles.tile([K, so_piece], bf16, tag=f"eo2_sb{pi}")
                nc.sync.dma_start(out=e_t, in_=eo2[:, psl])
                eo2_sbs.append(e_t)

            def load_iou(ci):
                sl = slice(ci * C, (ci + 1) * C)
                t = io.tile([M_loc, C], bf16, tag="iouc")
                nc.sync.dma_start(out=t, in_=iou[:, sl])
                iou_tiles.append(t)

            # interleave so the first chunk's operands land first
            for ci in range(NCH):
                load_piece(ci)
                load_iou(ci)

            accL = singles.tile([M_loc, 2 * NCH], f32)
            NCH_DVE = max(0, min(NCH, (6 * NCH) // 16))  # count chunks on DVE
            NCH_ACT = NCH - NCH_DVE                      # count chunks on ACT (Sign)
            accM = singles.tile([M_loc, max(NCH_DVE, 1)], f32)
            accS = singles.tile([M_loc, max(NCH_ACT, 1)], f32)
            neg_half = singles.tile([M_loc, 1], f32)
            nc.vector.memset(neg_half, -IOU_THRESHOLD)
            # fixed throwaway output tiles: same-engine WAW ordering only,
            # so no cross-engine release semaphores per chunk
            junk_dve = singles.tile([M_loc, 2 * C], bf16, tag="junk_dve")
            junk_act = singles.tile([M_loc, C], bf16, tag="junk_act")

            for ci in range(NCH):
                iouc = iou_tiles[ci]

                so2rep = psum.tile([M_loc, C], f32, tag="ps_s")
                eo2rep = psum.tile([M_loc, C], f32, tag="ps_e")
                for mi in range(MMW):
                    psl = slice(mi * 512, (mi + 1) * 512)
                    nc.tensor.matmul(
                        so2rep[:, psl], lhsT=R_sb,
                        rhs=so2_sbs[ci][:, psl],
                        start=True, stop=True,
                    )
                for mi in range(MMW):
                    psl = slice(mi * 512, (mi + 1) * 512)
                    nc.tensor.matmul(
                        eo2rep[:, psl], lhsT=R_sb,
                        rhs=eo2_sbs[ci][:, psl],
                        start=True, stop=True,
                    )

                ab = work.tile([M_loc, 2, C], bf16, tag="ab")
                nc.scalar.activation(
                    out=ab[:, 0, :],
                    in_=so2rep,
                    func=mybir.ActivationFunctionType.Abs,
                    bias=ntgt_sb[:, 0:1],
                    scale=1.0,
                )
                nc.scalar.activation(
                    out=ab[:, 1, :],
                    in_=eo2rep,
                    func=mybir.ActivationFunctionType.Abs,
                    bias=ntgt_sb[:, 1:2],
                    scale=1.0,
                )

                # NOTE: offloading an op to GPSIMD is a net loss here - GpSimd
                # and DVE share SBUF ports (exclusive lock) and both engines
                # drop to half rate when streaming concurrently.
                nc.vector.scalar_tensor_tensor(
                    out=junk_dve[:, 0:C],
                    in0=iouc,
                    scalar=IOU_THRESHOLD,
                    in1=ab[:, 0, :],
                    op0=mybir.AluOpType.is_gt,
                    op1=mybir.AluOpType.mult,
                    accum_out=accL[:, ci : ci + 1],
                )
                nc.vector.scalar_tensor_tensor(
                    out=junk_dve[:, C : 2 * C],
                    in0=iouc,
                    scalar=IOU_THRESHOLD,
                    in1=ab[:, 1, :],
                    op0=mybir.AluOpType.is_gt,
                    op1=mybir.AluOpType.mult,
                    accum_out=accL[:, NCH + ci : NCH + ci + 1],
                )
                if ci < NCH_DVE:
                    # mask count on DVE (accum_out reduce op is op1)
                    nc.vector.tensor_scalar(
                        out=junk_dve[:, 0:C],
                        in0=iouc,
                        scalar1=IOU_THRESHOLD,
                        scalar2=None,
                        op0=mybir.AluOpType.is_gt,
                        op1=mybir.AluOpType.add,
                        accum_out=accM[:, ci : ci + 1],
                    )
                else:
                    # mask count on ACT: accum of Sign(iou-0.5). The host
                    # nudges bf16 iou off the exact 0.5 value in both
                    # directions, so sign is strictly +-1 and
                    # count = (accum + C) / 2 exactly.
                    nc.scalar.activation(
                        out=junk_act[:, 0:C],
                        in_=iouc,
                        func=mybir.ActivationFunctionType.Sign,
                        bias=neg_half[:, 0:1],
                        scale=1.0,
                        accum_out=accS[:, ci - NCH_DVE : ci - NCH_DVE + 1],
                    )

            outsb = singles.tile([M_loc, 2], f32)
            nc.vector.reduce_sum(
                out=outsb[:, 0:1], in_=accL, axis=mybir.AxisListType.X
            )
            # count = sum(accM) + (sum(accS) + NCH_ACT*C)/2
            cnt_m = singles.tile([M_loc, 1], f32)
            if NCH_DVE > 0:
                nc.vector.reduce_sum(out=cnt_m, in_=accM, axis=mybir.AxisListType.X)
            else:
                nc.vector.memset(cnt_m, 0.0)
            cnt_s = singles.tile([M_loc, 1], f32)
            if NCH_ACT > 0:
                nc.vector.reduce_sum(out=cnt_s, in_=accS, axis=mybir.AxisListType.X)
            else:
                nc.vector.memset(cnt_s, 0.0)
            cnt_s2 = singles.tile([M_loc, 1], f32)
            nc.vector.tensor_scalar(
                out=cnt_s2,
                in0=cnt_s,
                scalar1=0.5,
                scalar2=float(NCH_ACT * C) / 2.0,
                op0=mybir.AluOpType.mult,
                op1=mybir.AluOpType.add,
            )
            nc.vector.tensor_tensor(
                out=outsb[:, 1:2], in0=cnt_m, in1=cnt_s2,
                op=mybir.AluOpType.add,
            )
            nc.sync.dma_start(out=out[:, :], in_=outsb)

    nc.compile()
    return nc


def _scatter_m2s(num_targets, S, M):
    """target index -> source video index, mirroring jnp.repeat(
    arange(S), num_targets, total_repeat_length=M)."""
    cum = np.cumsum(num_targets.astype(np.int64))
    idx = np.searchsorted(cum, np.arange(M), side="right")
    return np.clip(idx, 0, S - 1).astype(np.int64)


def _numpy_reference(start_offset, end_offset, tgt_moments, num_targets, iou2ds, mask2d):
    """Exact numpy replica of reference.py (topk fallback path)."""
    M, N, _ = iou2ds.shape
    S, P = start_offset.shape
    scatter = _scatter_m2s(num_targets, S, M)
    so = start_offset[scatter]
    eo = end_offset[scatter]
    r, c = np.nonzero(mask2d)
    if r.shape[0] < P:
        pad = P - r.shape[0]
        r = np.concatenate([r, np.zeros(pad, dtype=r.dtype)])
        c = np.concatenate([c, np.zeros(pad, dtype=c.dtype)])
    else:
        r, c = r[:P], c[:P]
    iou1 = iou2ds.reshape(M, N * N)[:, r * N + c]
    # top-k scatter mask + threshold mask
    topk_idx = np.argsort(-iou1, axis=1, kind="stable")[:, :TOPK]
    mask = np.zeros((M, P), dtype=np.float32)
    np.put_along_axis(mask, topk_idx, 1.0, axis=1)
    mask = np.where(iou1 > IOU_THRESHOLD, np.float32(1.0), mask)
    starts = (r.astype(np.float32) / N)[None, :]
    ends = ((c.astype(np.float32) + 1.0) / N)[None, :]
    sot = tgt_moments[:, 0:1] - starts
    eot = tgt_moments[:, 1:2] - ends
    loss = np.abs(so - sot) + np.abs(eo - eot)
    return np.float32((loss * mask).sum(dtype=np.float64) / mask.sum(dtype=np.float64))


def kernel(**inputs):
    global LAST_EXEC_TIME_NS, LAST_RESULTS
    _ensure_ntff_hook()
    import ml_dtypes

    from concourse.bass_utils import run_bass_kernel_spmd

    start_offset = np.asarray(inputs["start_offset"], dtype=np.float32)
    end_offset = np.asarray(inputs["end_offset"], dtype=np.float32)
    tgt_moments = np.asarray(inputs["tgt_moments"], dtype=np.float32)
    num_targets = np.asarray(inputs["num_targets"])
    iou2ds = np.asarray(inputs["iou2ds"], dtype=np.float32)
    mask2d = np.asarray(inputs["mask2d"])

    bf16 = ml_dtypes.bfloat16

    M, N, _ = iou2ds.shape
    S, P = start_offset.shape
    assert M % N_CORES == 0
    M_loc = M // N_CORES

    # proposal-grid constants from mask2d (row-major nonzero, padded like jnp)
    r, c = np.nonzero(mask2d)
    if r.shape[0] < P:
        pad = P - r.shape[0]
        r = np.concatenate([r, np.zeros(pad, dtype=r.dtype)])
        c = np.concatenate([c, np.zeros(pad, dtype=c.dtype)])
    else:
        r, c = r[:P], c[:P]
    starts = r.astype(np.float32) / np.float32(N)
    ends = (c.astype(np.float32) + np.float32(1.0)) / np.float32(N)

    # iou1ds = iou2ds[:, r, c]; identity reshape when mask2d is all ones
    flat_idx = r.astype(np.int64) * N + c.astype(np.int64)
    iou_flat = iou2ds.reshape(M, N * N)
    if not (flat_idx == np.arange(P)).all():
        iou_flat = np.ascontiguousarray(iou_flat[:, flat_idx])
    # bf16 halves the iou DMA bytes, but values that round exactly onto the
    # 0.5 threshold would corrupt the comparison. Nudge those one bf16 ulp
    # away from 0.5 in the direction of their f32 value; this makes
    # (iou_bf16 > 0.5) == (iou_f32 > 0.5) for every element AND leaves no
    # element exactly at 0.5, so the device's Sign(iou-0.5) count path is
    # strictly +-1 (exact counts).
    iou_bf16 = iou_flat.astype(bf16)
    on_thr = iou_bf16 == bf16(IOU_THRESHOLD)
    above = on_thr & (iou_flat > np.float32(IOU_THRESHOLD))
    below = on_thr & ~above
    if above.any():
        iou_bf16[above] = bf16(0.50390625)  # nextafter(0.5, up) in bf16
    if below.any():
        iou_bf16[below] = bf16(0.498046875)  # nextafter(0.5, down) in bf16

    # fold grid constants into the offsets: loss_a = |so2 - tgt_s|
    so2_full = (start_offset + starts[None, :]).astype(bf16)
    eo2_full = (end_offset + ends[None, :]).astype(bf16)

    # per-core source-row windows + replication matrices
    scatter = _scatter_m2s(num_targets, S, M)
    src_lo = np.empty(N_CORES, dtype=np.int64)
    n_src = np.empty(N_CORES, dtype=np.int64)
    for core in range(N_CORES):
        seg = scatter[core * M_loc : (core + 1) * M_loc]
        src_lo[core] = seg[0]
        n_src[core] = seg[-1] - seg[0] + 1
    K = int(n_src.max())

    in_maps = []
    for core in range(N_CORES):
        seg = scatter[core * M_loc : (core + 1) * M_loc]
        lo = int(src_lo[core])
        so2_c = np.zeros((K, P), dtype=bf16)
        eo2_c = np.zeros((K, P), dtype=bf16)
        hi = min(lo + K, S)
        so2_c[: hi - lo] = so2_full[lo:hi]
        eo2_c[: hi - lo] = eo2_full[lo:hi]
        repl = np.zeros((K, M_loc), dtype=bf16)
        repl[seg - lo, np.arange(M_loc)] = 1.0
        ntgt = np.ascontiguousarray(
            -tgt_moments[core * M_loc : (core + 1) * M_loc, :]
        ).astype(np.float32)
        in_maps.append(
            {
                "iou": np.ascontiguousarray(iou_bf16[core * M_loc : (core + 1) * M_loc]),
                "so2": so2_c,
                "eo2": eo2_c,
                "repl": repl,
                "ntgt": ntgt,
            }
        )

    cache_key = (K, M_loc, P)
    if cache_key not in _NC_CACHE:
        _NC_CACHE[cache_key] = _build_nc(K, M_loc, P, C=1024)
    nc = _NC_CACHE[cache_key]

    res = run_bass_kernel_spmd(nc, in_maps, list(range(N_CORES)))
    LAST_EXEC_TIME_NS = res.exec_time_ns
    LAST_RESULTS = res

    loss_sum = 0.0
    mask_sum = 0.0
    min_count = np.inf
    for core in range(N_CORES):
        part = res.results[core]["out"]  # [M_loc, 2]
        loss_sum += part[:, 0].sum(dtype=np.float64)
        mask_sum += part[:, 1].sum(dtype=np.float64)
        min_count = min(min_count, part[:, 1].min())

    if min_count < TOPK:
        # some row's top-k reaches below the threshold: the threshold mask is
        # not exact there -> use the exact (slow) host path
        return _numpy_reference(
            start_offset, end_offset, tgt_moments, num_targets, iou2ds, mask2d
        )

    return np.float32(loss_sum / mask_sum)

